# revision 35
# baseline (speedup 1.0000x reference)
"""Trainium2 Bass kernel for a 3-sublayer decoder block (nn_DecoderLayer).

Reference computation (B=2, S=2048, D=1024, H=16, DK=64, FF=4096, fp32):
  sa = causal_mha(x, x)          ; x1 = seqnorm(sa + x)
  ca = mha(x1, enc)              ; x2 = seqnorm(ca + x1)
  ffn = relu(x2 @ W1 + b1) @ W2 + b2 ; out = seqnorm(ffn + x2)
seqnorm normalizes over the SEQUENCE dim and divides by the unbiased VARIANCE
(reference quirk); attention has no output projection.

Sharding (8 cores, one replica group): every core processes BOTH batch
elements; heads are split 2-per-core (=> a 128-wide channel slice of every
sublayer output per batch) and the FF hidden dim is split 512-per-core
(Megatron column/row split). seqnorm (over S) stays fully local.

v2 schedule: collectives are split PER BATCH and software-pipelined against
the other batch's compute: AG(x1,b0) runs under attn1(b1), AG(x2,b0) under
attn2(b1), RS(b0) under FFN(b1), so only the tail RS is exposed. Encoder K/V
projections are interleaved into the self-attention chunk loop (PE-heavy work
under the ACT-bound softmax) to keep the PE HAM-warm. All matmul inputs and
collective payloads are bf16 (fp32 PSUM accumulation, fp32 seqnorm); rel-err
budget is 2e-2.

All activations live TRANSPOSED on-chip ([d, s] layout): weights feed the PE
stationary port as stored, seqnorm reduces along the free axis, and attention
computes E^T = exp(K @ Q^T) tiles directly ([sk, sq]) with no transposes in
the main path (V is produced transposed like Q/K and flipped back via cheap
PE-transposes). Softmax denominators come from an appended ones-column on V
(row 64 of the PV psum).
"""

import os
import sys

import numpy as np

for _p in ("/opt/trn_rl_repo", "/root/.axon_site/_ro/trn_rl_repo"):
    if _p not in sys.path and os.path.isdir(_p):
        sys.path.append(_p)

import concourse.bass as bass
import concourse.mybir as mybir
import concourse.tile as tile
from concourse import bacc
from concourse.bass import ts
from concourse.bass_utils import run_bass_kernel_spmd

F32 = mybir.dt.float32
F32R = mybir.dt.float32r
BF16 = mybir.dt.bfloat16
AF = mybir.ActivationFunctionType
ALU = mybir.AluOpType

B, S, D, H = 2, 2048, 1024, 16
DK = D // H            # 64
FF = 4 * D             # 4096
NCORES = 8
HL = H // NCORES       # 2 heads per core
DL = DK * HL           # 128 channels per core
FFL = FF // NCORES     # 512 ff dims per core
KC = D // 128          # 8 contraction chunks of the full model dim
FCL = FFL // 128       # 4 local ff chunks
ST = S // 512          # 4 sequence tiles of 512
SC = S // 128          # 16 sequence chunks of 128
SCALE = 1.0 / np.sqrt(DK)
VARF = (S - 1) / S     # unbiased-variance factor applied to 1/var_pop

RG = [[0, 1, 2, 3, 4, 5, 6, 7]]

_CACHED_NC = None


def _build():
    nc = bacc.Bacc("TRN2", target_bir_lowering=False, debug=False,
                   num_devices=NCORES)

    # ---- per-core external inputs ----
    # activations in [p, b, t, c, j] tile layout so each (b,t) xs DMA is one
    # contiguous 8KB-per-partition read
    xTd = nc.dram_tensor("xTd", [128, B, ST, KC, 512], BF16,
                         kind="ExternalInput")
    encd = nc.dram_tensor("encd", [128, B, ST, KC, 512], BF16,
                          kind="ExternalInput")
    resd = nc.dram_tensor("resd", [128, B, S], F32, kind="ExternalInput")
    wq1d = nc.dram_tensor("wq1d", [128, KC, DL], BF16, kind="ExternalInput")
    wk1d = nc.dram_tensor("wk1d", [128, KC, DL], BF16, kind="ExternalInput")
    wv1d = nc.dram_tensor("wv1d", [128, KC, DL], BF16, kind="ExternalInput")
    wq2d = nc.dram_tensor("wq2d", [128, KC, DL], BF16, kind="ExternalInput")
    wk2d = nc.dram_tensor("wk2d", [128, KC, DL], BF16, kind="ExternalInput")
    wv2d = nc.dram_tensor("wv2d", [128, KC, DL], BF16, kind="ExternalInput")
    w1d = nc.dram_tensor("w1d", [128, KC, FFL], BF16, kind="ExternalInput")
    w2d = nc.dram_tensor("w2d", [128, FCL, D], BF16, kind="ExternalInput")
    bqkd = nc.dram_tensor("bqkd", [128, 4], F32, kind="ExternalInput")
    bvd = nc.dram_tensor("bvd", [128, 2], F32, kind="ExternalInput")
    b1d = nc.dram_tensor("b1d", [128, FCL], F32, kind="ExternalInput")
    b2d = nc.dram_tensor("b2d", [128, 1], F32, kind="ExternalInput")

    outT = nc.dram_tensor("outT", [B * DL, S], F32, kind="ExternalOutput")
    DBG = bool(os.environ.get("BASSDBG"))
    if DBG:
        dbg_x1 = nc.dram_tensor("dbg_x1", [128, B, S], F32,
                                kind="ExternalOutput")
        dbg_x2 = nc.dram_tensor("dbg_x2", [128, B, S], F32,
                                kind="ExternalOutput")
        dbg_kT1 = nc.dram_tensor("dbg_kT1", [128, B, S], BF16,
                                 kind="ExternalOutput")
        dbg_vO1 = nc.dram_tensor("dbg_vO1", [128, B, SC, HL, DK + 1], F32,
                                 kind="ExternalOutput")
        dbg_q1 = nc.dram_tensor("dbg_q1", [128, B, ST, 512], BF16,
                                kind="ExternalOutput")

    def rview(t):   # [8*128, s] -> [p, rank, s]
        return t[:].rearrange("(r p) s -> p r s", r=NCORES)

    with tile.TileContext(nc) as tc:
        import contextlib
        ctx = contextlib.ExitStack()
        with ctx:
            sb = ctx.enter_context(tc.tile_pool(name="sb", bufs=1))
            dram = ctx.enter_context(tc.tile_pool(name="dr", bufs=1,
                                                  space="DRAM"))
            pp = ctx.enter_context(tc.tile_pool(name="pp", bufs=2,
                                                space="PSUM"))
            pe = ctx.enter_context(tc.tile_pool(name="pe", bufs=2,
                                                space="PSUM"))
            pz = ctx.enter_context(tc.tile_pool(name="pz", bufs=2,
                                                space="PSUM"))

            # collective bounce buffers, one set per batch
            x1b = [dram.tile([DL, S], BF16, tag=f"x1b{b}", name=f"x1b{b}")
                   for b in range(B)]
            x1f = [dram.tile([NCORES * DL, S], BF16, tag=f"x1f{b}",
                             name=f"x1f{b}") for b in range(B)]
            x2b = [dram.tile([DL, S], BF16, tag=f"x2b{b}", name=f"x2b{b}")
                   for b in range(B)]
            x2f = [dram.tile([NCORES * DL, S], BF16, tag=f"x2f{b}",
                             name=f"x2f{b}") for b in range(B)]
            # FFN partial-sum RS buffers, per (batch, seq-tile)
            rsi = [[dram.tile([NCORES * DL, 512], BF16, tag=f"rsi{b}{t}",
                              name=f"rsi{b}{t}") for t in range(ST)]
                   for b in range(B)]
            rso = [[dram.tile([DL, 512], BF16, tag=f"rso{b}{t}",
                              name=f"rso{b}{t}") for t in range(ST)]
                   for b in range(B)]

            # --- small persistent tiles ---
            bqk_sb = sb.tile([128, 4], F32, tag="bias", bufs=1)
            nc.sync.dma_start(out=bqk_sb, in_=bqkd[:])
            bvf_sb = sb.tile([128, 2], F32, tag="bias2", bufs=1)
            nc.sync.dma_start(out=bvf_sb, in_=bvd[:])
            b1_sb = sb.tile([128, FCL], F32, tag="bias3", bufs=1)
            nc.sync.dma_start(out=b1_sb, in_=b1d[:])
            b2_sb = sb.tile([128, 1], F32, tag="bias4", bufs=1)
            nc.sync.dma_start(out=b2_sb, in_=b2d[:])

            # identity for PE transposes
            idb = sb.tile([128, 128], BF16, tag="idb", bufs=1)
            nc.vector.memset(idb, 1.0)
            nc.gpsimd.affine_select(out=idb, in_=idb,
                                    compare_op=ALU.is_equal, fill=0.0,
                                    base=0, channel_multiplier=-1,
                                    pattern=[[1, 128]])

            def load_w(dram_t, name, cols):
                w = sb.tile([128, KC, cols], BF16, tag=f"w_{name}", bufs=1,
                            name=name)
                nc.sync.dma_start(out=w, in_=dram_t[:])
                return w

            wq1 = load_w(wq1d, "wq1", DL)
            wk1 = load_w(wk1d, "wk1", DL)
            wv1 = load_w(wv1d, "wv1", DL)

            # persistent activation tiles
            kT1 = sb.tile([128, B, S], BF16, tag="kT1", bufs=1, name="kT1")
            vO1 = sb.tile([128, B, SC, HL, DK + 1], F32R, tag="vO1", bufs=1,
                          name="vO1")
            kT2 = sb.tile([128, B, S], BF16, tag="kT2", bufs=1, name="kT2")
            vO2 = sb.tile([128, B, SC, HL, DK + 1], F32R, tag="vO2", bufs=1,
                          name="vO2")
            qtsA = sb.tile([128, B, ST, 512], BF16, tag="qtsA", bufs=1,
                           name="qtsA")
            qtsC = sb.tile([128, B, ST, 512], BF16, tag="qtsC", bufs=1,
                           name="qtsC")
            # x ring: x1, x2, x3 (x3 reuses x1's buffer)
            x1 = sb.tile([128, B, S], F32, tag="xl", bufs=2, name="x1")
            x2 = sb.tile([128, B, S], F32, tag="xl", bufs=2, name="x2")

            def proj128(psrc, w, bias_col, out_ap):
                """One [128, 512] projection psum: out = W.T @ x (+bias)."""
                ps = pp.tile([128, 512], F32, tag="pp", name="ps")
                for k in range(KC):
                    nc.tensor.matmul(ps, w[:, k, :], psrc[:, k, :],
                                     start=(k == 0), stop=(k == KC - 1))
                nc.vector.tensor_scalar(out=out_ap, in0=ps,
                                        scalar1=bqk_sb[:, bias_col:bias_col + 1],
                                        scalar2=None, op0=ALU.add)

            def vproj(xs, wv, b, t, vO):
                """v^T projection + PE-transpose back to v-normal layout with
                an appended ones column."""
                vt = sb.tile([128, 512], BF16, tag="vt", bufs=2, name="vt")
                ps = pp.tile([128, 512], F32, tag="pp", name="ps")
                for k in range(KC):
                    nc.tensor.matmul(ps, wv[:, k, :], xs[:, k, :],
                                     start=(k == 0), stop=(k == KC - 1))
                nc.vector.tensor_copy(out=vt, in_=ps)
                for sc in range(4):
                    c = 4 * t + sc
                    tp = pp.tile([128, 1024], BF16, tag="pp", name="tp")
                    nc.tensor.transpose(tp[:, 0:128], vt[:, ts(sc, 128)], idb)
                    for h in range(HL):
                        nc.vector.tensor_copy(
                            out=vO[:, b, c, h, 0:DK],
                            in_=tp[:, ts(h, DK)])
                    nc.vector.tensor_scalar(
                        out=vO[:, b, c, :, DK:DK + 1],
                        in0=tp[:, 0:HL][:, :, None],
                        scalar1=0.0, scalar2=1.0,
                        op0=ALU.mult, op1=ALU.add)

            def qkv1_tile(b, t):
                xs = sb.tile([128, KC, 512], BF16, tag="xs", bufs=3,
                             name="xs")
                nc.sync.dma_start(out=xs, in_=xTd.ap()[:, b, t])
                proj128(xs, wq1, 0, qtsA[:, b, t, :])
                proj128(xs, wk1, 1, kT1[:, b, ts(t, 512)])
                vproj(xs, wv1, b, t, vO1)

            def kv2_tile(b, t):
                es = sb.tile([128, KC, 512], BF16, tag="xs", bufs=3,
                             name="es")
                nc.sync.dma_start(out=es, in_=encd.ap()[:, b, t])
                proj128(es, wk2, 3, kT2[:, b, ts(t, 512)])
                vproj(es, wv2, b, t, vO2)

            def attn_one(b, t, qt, kT, vO, xout, resid_sb, bv_col, causal,
                         stats=None):
                """One (b, sq-tile): E^T chunk tiles for both heads packed in
                one 2-bank psum (adjacent matmuls hit different PE row-groups
                and overlap), PV with ones-row, then a fused both-heads
                normalize + bias + residual into xout. The chunk loop is
                software-pipelined so E(c+1) sits AHEAD of PV(c) in the
                in-order PE queue and runs under exp(c). resid_sb is an SBUF
                [128, B, S] tile (or None to DMA the residual from resd)."""
                nchunks = (4 * t + 4) if causal else SC
                zps = [pz.tile([128, 512], F32, tag="pz", name="zps")
                       for _ in range(HL)]
                if resid_sb is None:
                    rs_ = sb.tile([128, 512], F32, tag="rs", bufs=2,
                                  name="rs_")
                    nc.sync.dma_start(out=rs_,
                                      in_=resd.ap()[:, b, ts(t, 512)])
                    resid_ap = rs_[:, :]
                else:
                    resid_ap = resid_sb[:, b, ts(t, 512)]

                def emit_E(c):
                    eps = pe.tile([128, 1024], F32, tag="pe", name="eps")
                    for h in range(HL):
                        hb = h * 64
                        nc.tensor.matmul(
                            eps[:, ts(h, 512)],
                            kT[hb:hb + 64, b, ts(c, 128)],
                            qt[hb:hb + 64, :],
                            start=True, stop=True)
                    return eps

                def emit_exp(c, eps):
                    et = sb.tile([128, 1024], F32R, tag="E", bufs=2,
                                 name="et")
                    nc.scalar.activation(out=et, in_=eps, func=AF.Exp,
                                         scale=float(SCALE))
                    if causal and c >= 4 * t:
                        j = c - 4 * t
                        for h in range(HL):
                            nc.gpsimd.affine_select(
                                out=et[:, ts(h, 512)],
                                in_=et[:, ts(h, 512)],
                                compare_op=ALU.is_ge,
                                fill=0.0, base=-(j * 128),
                                channel_multiplier=-1,
                                pattern=[[1, 512]])
                    return et

                def emit_PV(c, et):
                    for h in range(HL):
                        nc.tensor.matmul(
                            zps[h][0:DK + 1, :],
                            vO[:, b, c, h, :],
                            et[:, ts(h, 512)],
                            start=(c == 0),
                            stop=(c == nchunks - 1))

                et_prev = emit_exp(0, emit_E(0))
                for c in range(1, nchunks):
                    eps_c = emit_E(c)
                    emit_PV(c - 1, et_prev)
                    et_prev = emit_exp(c, eps_c)
                emit_PV(nchunks - 1, et_prev)
                # copy z (both heads) + denominators out of PSUM promptly so
                # the pz ring frees for the next row's PV accumulation
                zc = sb.tile([128, 512], F32, tag="zc", bufs=2, name="zc")
                dns = [sb.tile([1, 512], F32, tag="dn", bufs=4, name="dn")
                       for _ in range(HL)]
                for h in range(HL):
                    nc.vector.tensor_copy(out=zc[ts(h, 64), :],
                                          in_=zps[h][0:DK, :])
                    nc.vector.tensor_copy(out=dns[h],
                                          in_=zps[h][DK:DK + 1, :])
                # partition_broadcast mis-addresses outputs with a non-zero
                # base partition, so head 1 goes via a base-0 scratch tile
                rb = sb.tile([128, 512], F32, tag="rb", bufs=2, name="rb")
                nc.gpsimd.partition_broadcast(out_ap=rb[0:64, :],
                                              in_ap=dns[0])
                rb2 = sb.tile([64, 512], F32, tag="rb2", bufs=2, name="rb2")
                nc.gpsimd.partition_broadcast(out_ap=rb2, in_ap=dns[1])
                nc.vector.tensor_copy(out=rb[64:128, :], in_=rb2)
                nc.vector.reciprocal(rb, rb)
                nc.vector.tensor_mul(zc, zc, rb)
                nc.vector.scalar_tensor_tensor(
                    out=xout[:, b, ts(t, 512)], in0=zc,
                    scalar=bvf_sb[:, bv_col:bv_col + 1],
                    op0=ALU.add, in1=resid_ap, op1=ALU.add)
                if stats is not None:
                    nc.vector.bn_stats(out=stats[:, t, :],
                                       in_=xout[:, b, ts(t, 512)])

            def seqnorm_b(xt, b, stats=None):
                """In-place sequence-norm of one batch of a [128, B, S] f32
                tile. If per-tile bn_stats were precomputed, pass them in."""
                if stats is None:
                    stats = sb.tile([128, ST, 6], F32, tag="bnst", bufs=2,
                                    name="stats")
                    for g in range(ST):
                        nc.vector.bn_stats(out=stats[:, g, :],
                                           in_=xt[:, b, ts(g, 512)])
                mv = sb.tile([128, 2], F32, tag="bnmv", bufs=2, name="mv")
                nc.vector.bn_aggr(out=mv, in_=stats)
                r = sb.tile([128, 1], F32, tag="bnr", bufs=2, name="r")
                nc.vector.reciprocal(r, mv[:, 1:2])
                nc.vector.tensor_scalar(out=r, in0=r, scalar1=float(VARF),
                                        scalar2=None, op0=ALU.mult)
                mr = sb.tile([128, 1], F32, tag="bnmr", bufs=2, name="mr")
                nc.vector.scalar_tensor_tensor(
                    out=mr, in0=mv[:, 0:1], scalar=-1.0, op0=ALU.mult,
                    in1=r, op1=ALU.mult)
                nc.vector.scalar_tensor_tensor(
                    out=xt[:, b, :], in0=xt[:, b, :], scalar=r,
                    op0=ALU.mult, in1=mr.to_broadcast((128, S)),
                    op1=ALU.add)

            def bounce_ag(xt, b, bnc, full):
                """seqnorm'd batch b of xt -> bf16 bounce -> AllGather."""
                xc = sb.tile([128, S], BF16, tag="xc", bufs=2, name="xc")
                nc.vector.tensor_copy(out=xc, in_=xt[:, b, :])
                nc.sync.dma_start(out=bnc[:], in_=xc)
                nc.gpsimd.collective_compute(
                    "AllGather", ALU.bypass, replica_groups=RG,
                    ins=[bnc[:]], outs=[full[:]])

            # ============ sublayer 1 + encoder K/V (interleaved) ===========
            wq2 = wk2 = wv2 = w1 = w2 = None
            for b in range(B):
                for t in range(ST):
                    qkv1_tile(b, t)
                if b == 0:
                    # big later-phase weight loads sit behind the first xs
                    # loads on the DMA queue so they don't delay the start
                    wk2 = load_w(wk2d, "wk2", DL)
                    wv2 = load_w(wv2d, "wv2", DL)
                    wq2 = load_w(wq2d, "wq2", DL)
                    w1 = sb.tile([128, KC, FFL], BF16, tag="w_w1", bufs=1,
                                 name="w1")
                    nc.sync.dma_start(out=w1, in_=w1d[:])
                    w2 = sb.tile([128, FCL, D], BF16, tag="w_w2", bufs=1,
                                 name="w2")
                    nc.sync.dma_start(out=w2, in_=w2d[:])
                stats1 = sb.tile([128, ST, 6], F32, tag="bnst", bufs=2,
                                 name="stats1")
                for t in range(ST):
                    attn_one(b, t, qtsA[:, b, t, :], kT1, vO1, x1,
                             None, bv_col=0, causal=True, stats=stats1)
                    kv2_tile(b, t)
                seqnorm_b(x1, b, stats1)
                bounce_ag(x1, b, x1b[b], x1f[b])

            if DBG:
                for b in range(B):
                    nc.sync.dma_start(out=dbg_x1[:, b, :], in_=x1[:, b, :])
                nc.sync.dma_start(out=dbg_kT1[:], in_=kT1)
                nc.sync.dma_start(out=dbg_vO1[:], in_=vO1.bitcast(F32))
                nc.sync.dma_start(out=dbg_q1[:], in_=qtsA)

            # ================= sublayer 2: cross-attention =================
            for b in range(B):
                x1f_v = rview(x1f[b])
                for t in range(ST):
                    xs = sb.tile([128, KC, 512], BF16, tag="xs", bufs=3,
                                 name="xs")
                    nc.sync.dma_start(out=xs, in_=x1f_v[:, :, ts(t, 512)])
                    proj128(xs, wq2, 2, qtsC[:, b, t, :])
                stats2 = sb.tile([128, ST, 6], F32, tag="bnst", bufs=2,
                                 name="stats2")
                for t in range(ST):
                    attn_one(b, t, qtsC[:, b, t, :], kT2, vO2, x2,
                             x1, bv_col=1, causal=False, stats=stats2)
                seqnorm_b(x2, b, stats2)
                bounce_ag(x2, b, x2b[b], x2f[b])

            if DBG:
                for b in range(B):
                    nc.sync.dma_start(out=dbg_x2[:, b, :], in_=x2[:, b, :])

            # ================= sublayer 3: FFN =============================
            for b in range(B):
                x2f_v = rview(x2f[b])
                for t in range(ST):
                    xs = sb.tile([128, KC, 512], BF16, tag="xs", bufs=3,
                                 name="xs")
                    nc.sync.dma_start(out=xs, in_=x2f_v[:, :, ts(t, 512)])
                    hT = sb.tile([128, FCL, 512], BF16, tag="hT", bufs=2,
                                 name="hT")
                    for fc in range(FCL):
                        ps_h = pp.tile([128, 512], F32, tag="pp",
                                       name="ps_h")
                        for k in range(KC):
                            nc.tensor.matmul(ps_h,
                                             w1[:, k, ts(fc, 128)],
                                             xs[:, k, :],
                                             start=(k == 0),
                                             stop=(k == KC - 1))
                        nc.vector.tensor_scalar(
                            out=hT[:, fc, :], in0=ps_h,
                            scalar1=b1_sb[:, fc:fc + 1], scalar2=0.0,
                            op0=ALU.add, op1=ALU.max)
                    rsi_v = rview(rsi[b][t])
                    for ec in range(KC):
                        ps_y = pp.tile([128, 512], F32, tag="pp",
                                       name="ps_y")
                        for fc in range(FCL):
                            nc.tensor.matmul(ps_y,
                                             w2[:, fc, ts(ec, 128)],
                                             hT[:, fc, :],
                                             start=(fc == 0),
                                             stop=(fc == FCL - 1))
                        ys = sb.tile([128, 512], BF16, tag="ys", bufs=3,
                                     name="ys")
                        nc.vector.tensor_copy(out=ys, in_=ps_y)
                        nc.sync.dma_start(out=rsi_v[:, ec, :], in_=ys)
                    nc.gpsimd.collective_compute(
                        "ReduceScatter", ALU.add, replica_groups=RG,
                        ins=[rsi[b][t][:]], outs=[rso[b][t][:]])

            # y + b2 + x2 residual (per quarter, overlapping the RS stream),
            # seqnorm, write out
            x3 = sb.tile([128, B, S], F32, tag="xl", bufs=2, name="x3")
            outv = outT[:].rearrange("(b p) s -> p b s", p=128)
            for b in range(B):
                stats3 = sb.tile([128, ST, 6], F32, tag="bnst", bufs=2,
                                 name="stats3")
                for t in range(ST):
                    sl = ts(t, 512)
                    r3 = sb.tile([128, 512], BF16, tag="r3", bufs=2,
                                 name="r3")
                    nc.sync.dma_start(out=r3, in_=rso[b][t][:])
                    nc.vector.tensor_copy(out=x3[:, b, sl], in_=r3)
                    nc.vector.scalar_tensor_tensor(
                        out=x3[:, b, sl], in0=x3[:, b, sl],
                        scalar=b2_sb[:, 0:1], op0=ALU.add,
                        in1=x2[:, b, sl], op1=ALU.add)
                    nc.vector.bn_stats(out=stats3[:, t, :],
                                       in_=x3[:, b, sl])
                seqnorm_b(x3, b, stats3)
                nc.sync.dma_start(out=outv[:, b, :], in_=x3[:, b, :])

    nc.compile()
    return nc


def _get_nc():
    global _CACHED_NC
    if _CACHED_NC is None:
        _CACHED_NC = _build()
    return _CACHED_NC


def _bf16(a):
    import ml_dtypes
    return np.asarray(a, np.float32).astype(ml_dtypes.bfloat16)


def _chunked(a):
    """[D, N] -> [128, D//128, N] with [p, c, n] = a[128c+p, n]."""
    d, n = a.shape
    return np.ascontiguousarray(
        a.reshape(d // 128, 128, n).transpose(1, 0, 2))


def _tiled_act(xT):
    """[B, D, S] -> [128, B, ST, KC, 512] bf16 with
    [p, b, t, c, j] = x[b, 128c+p, 512t+j]."""
    a = xT.reshape(B, KC, 128, ST, 512).transpose(2, 0, 3, 1, 4)
    return np.ascontiguousarray(_bf16(a))


def _make_in_maps(decoder_input, encode_input,
                  Wq1, Wk1, Wv1, bq1, bk1, bv1,
                  Wq2, Wk2, Wv2, bq2, bk2, bv2,
                  W1, b1, W2, b2):
    xT = np.ascontiguousarray(
        np.transpose(np.asarray(decoder_input, np.float32), (0, 2, 1)))
    eT = np.ascontiguousarray(
        np.transpose(np.asarray(encode_input, np.float32), (0, 2, 1)))
    xTd_all = _tiled_act(xT)
    encd_all = _tiled_act(eT)
    in_maps = []
    for r in range(NCORES):
        hs = slice(DL * r, DL * (r + 1))
        fs = slice(FFL * r, FFL * (r + 1))
        resd = np.ascontiguousarray(
            xT[:, hs, :].transpose(1, 0, 2))          # [128, B, S]
        bqk_arr = np.stack([bq1[hs], bk1[hs], bq2[hs], bk2[hs]],
                           axis=1).astype(np.float32)  # [128, 4]
        bv_arr = np.stack([bv1[hs], bv2[hs]],
                          axis=1).astype(np.float32)   # [128, 2]
        in_maps.append({
            "xTd": xTd_all,
            "encd": encd_all,
            "resd": resd,
            "wq1d": _bf16(_chunked(np.ascontiguousarray(Wq1[:, hs]))),
            "wk1d": _bf16(_chunked(np.ascontiguousarray(Wk1[:, hs]))),
            "wv1d": _bf16(_chunked(np.ascontiguousarray(Wv1[:, hs]))),
            "wq2d": _bf16(_chunked(np.ascontiguousarray(Wq2[:, hs]))),
            "wk2d": _bf16(_chunked(np.ascontiguousarray(Wk2[:, hs]))),
            "wv2d": _bf16(_chunked(np.ascontiguousarray(Wv2[:, hs]))),
            "w1d": _bf16(_chunked(np.ascontiguousarray(W1[:, fs]))),
            "w2d": _bf16(_chunked(np.ascontiguousarray(W2[fs, :]))),
            "bqkd": bqk_arr,
            "bvd": bv_arr,
            "b1d": np.ascontiguousarray(
                b1[fs].reshape(FCL, 128).T.astype(np.float32)),
            "b2d": np.ascontiguousarray(
                b2[hs].reshape(128, 1).astype(np.float32)),
        })
    return in_maps


def kernel(**inputs):
    nc = _get_nc()
    in_maps = _make_in_maps(**{k: np.asarray(v) for k, v in inputs.items()})
    res = run_bass_kernel_spmd(nc, in_maps, core_ids=list(range(NCORES)),
                               trace=False)
    out = np.empty((B, S, D), np.float32)
    for r in range(NCORES):
        hs = slice(DL * r, DL * (r + 1))
        o = res.results[r]["outT"]                     # [B*DL, S]
        for b in range(B):
            out[b, :, hs] = o[b * DL:(b + 1) * DL].T
    return out


# revision 37
# speedup vs baseline: 1.1014x; 1.1014x over previous
"""Trainium2 Bass kernel for a 3-sublayer decoder block (nn_DecoderLayer).

Reference computation (B=2, S=2048, D=1024, H=16, DK=64, FF=4096, fp32):
  sa = causal_mha(x, x)          ; x1 = seqnorm(sa + x)
  ca = mha(x1, enc)              ; x2 = seqnorm(ca + x1)
  ffn = relu(x2 @ W1 + b1) @ W2 + b2 ; out = seqnorm(ffn + x2)
seqnorm normalizes over the SEQUENCE dim and divides by the unbiased VARIANCE
(reference quirk); attention has no output projection.

Sharding (8 cores, one replica group): every core processes BOTH batch
elements; heads are split 2-per-core (=> a 128-wide channel slice of every
sublayer output per batch) and the FF hidden dim is split 512-per-core
(Megatron column/row split). seqnorm (over S) stays fully local.

v2 schedule: collectives are split PER BATCH and software-pipelined against
the other batch's compute: AG(x1,b0) runs under attn1(b1), AG(x2,b0) under
attn2(b1), RS(b0) under FFN(b1), so only the tail RS is exposed. Encoder K/V
projections are interleaved into the self-attention chunk loop (PE-heavy work
under the ACT-bound softmax) to keep the PE HAM-warm. All matmul inputs and
collective payloads are bf16 (fp32 PSUM accumulation, fp32 seqnorm); rel-err
budget is 2e-2.

All activations live TRANSPOSED on-chip ([d, s] layout): weights feed the PE
stationary port as stored, seqnorm reduces along the free axis, and attention
computes E^T = exp(K @ Q^T) tiles directly ([sk, sq]) with no transposes in
the main path (V is produced transposed like Q/K and flipped back via cheap
PE-transposes). Softmax denominators come from an appended ones-column on V
(row 64 of the PV psum).
"""

import os
import sys

import numpy as np

for _p in ("/opt/trn_rl_repo", "/root/.axon_site/_ro/trn_rl_repo"):
    if _p not in sys.path and os.path.isdir(_p):
        sys.path.append(_p)

import concourse.bass as bass
import concourse.mybir as mybir
import concourse.tile as tile
from concourse import bacc
from concourse.bass import ts
from concourse.bass_utils import run_bass_kernel_spmd

F32 = mybir.dt.float32
F32R = mybir.dt.float32r
BF16 = mybir.dt.bfloat16
AF = mybir.ActivationFunctionType
ALU = mybir.AluOpType

B, S, D, H = 2, 2048, 1024, 16
DK = D // H            # 64
FF = 4 * D             # 4096
NCORES = 8
HL = H // NCORES       # 2 heads per core
DL = DK * HL           # 128 channels per core
FFL = FF // NCORES     # 512 ff dims per core
KC = D // 128          # 8 contraction chunks of the full model dim
FCL = FFL // 128       # 4 local ff chunks
ST = S // 512          # 4 sequence tiles of 512
SC = S // 128          # 16 sequence chunks of 128
SCALE = 1.0 / np.sqrt(DK)
VARF = (S - 1) / S     # unbiased-variance factor applied to 1/var_pop

RG = [[0, 1, 2, 3, 4, 5, 6, 7]]

_CACHED_NC = None


def _build():
    nc = bacc.Bacc("TRN2", target_bir_lowering=False, debug=False,
                   num_devices=NCORES)

    # ---- per-core external inputs ----
    # activations in [p, b, t, c, j] tile layout so each (b,t) xs DMA is one
    # contiguous 8KB-per-partition read
    xTd = nc.dram_tensor("xTd", [128, B, ST, KC, 512], BF16,
                         kind="ExternalInput")
    encd = nc.dram_tensor("encd", [128, B, ST, KC, 512], BF16,
                          kind="ExternalInput")
    resd = nc.dram_tensor("resd", [128, B, S], F32, kind="ExternalInput")
    wq1d = nc.dram_tensor("wq1d", [128, KC, DL], BF16, kind="ExternalInput")
    wk1d = nc.dram_tensor("wk1d", [128, KC, DL], BF16, kind="ExternalInput")
    wv1d = nc.dram_tensor("wv1d", [128, KC, DL], BF16, kind="ExternalInput")
    wq2d = nc.dram_tensor("wq2d", [128, KC, DL], BF16, kind="ExternalInput")
    wk2d = nc.dram_tensor("wk2d", [128, KC, DL], BF16, kind="ExternalInput")
    wv2d = nc.dram_tensor("wv2d", [128, KC, DL], BF16, kind="ExternalInput")
    w1d = nc.dram_tensor("w1d", [128, KC, FFL], BF16, kind="ExternalInput")
    w2d = nc.dram_tensor("w2d", [128, FCL, D], BF16, kind="ExternalInput")
    bqkd = nc.dram_tensor("bqkd", [128, 4], F32, kind="ExternalInput")
    bvd = nc.dram_tensor("bvd", [128, 2], F32, kind="ExternalInput")
    b1d = nc.dram_tensor("b1d", [128, FCL], F32, kind="ExternalInput")
    b2d = nc.dram_tensor("b2d", [128, 1], F32, kind="ExternalInput")

    outT = nc.dram_tensor("outT", [B * DL, S], F32, kind="ExternalOutput")
    DBG = bool(os.environ.get("BASSDBG"))
    if DBG:
        dbg_x1 = nc.dram_tensor("dbg_x1", [128, B, S], F32,
                                kind="ExternalOutput")
        dbg_x2 = nc.dram_tensor("dbg_x2", [128, B, S], F32,
                                kind="ExternalOutput")
        dbg_kT1 = nc.dram_tensor("dbg_kT1", [128, B, S], BF16,
                                 kind="ExternalOutput")
        dbg_vO1 = nc.dram_tensor("dbg_vO1", [128, B, SC, HL, DK + 1], F32,
                                 kind="ExternalOutput")
        dbg_q1 = nc.dram_tensor("dbg_q1", [128, B, ST, 512], BF16,
                                kind="ExternalOutput")

    def rview(t):   # [8*128, s] -> [p, rank, s]
        return t[:].rearrange("(r p) s -> p r s", r=NCORES)

    with tile.TileContext(nc) as tc:
        import contextlib
        ctx = contextlib.ExitStack()
        with ctx:
            sb = ctx.enter_context(tc.tile_pool(name="sb", bufs=1))
            dram = ctx.enter_context(tc.tile_pool(name="dr", bufs=1,
                                                  space="DRAM"))
            pp = ctx.enter_context(tc.tile_pool(name="pp", bufs=2,
                                                space="PSUM"))
            pe = ctx.enter_context(tc.tile_pool(name="pe", bufs=2,
                                                space="PSUM"))
            pz = ctx.enter_context(tc.tile_pool(name="pz", bufs=2,
                                                space="PSUM"))

            # collective bounce buffers, one set per batch
            x1b = [dram.tile([DL, S], BF16, tag=f"x1b{b}", name=f"x1b{b}")
                   for b in range(B)]
            x1f = [dram.tile([NCORES * DL, S], BF16, tag=f"x1f{b}",
                             name=f"x1f{b}") for b in range(B)]
            x2b = [dram.tile([DL, S], BF16, tag=f"x2b{b}", name=f"x2b{b}")
                   for b in range(B)]
            x2f = [dram.tile([NCORES * DL, S], BF16, tag=f"x2f{b}",
                             name=f"x2f{b}") for b in range(B)]
            # FFN partial-sum RS buffers, per (batch, seq-half)
            rsi = [[dram.tile([NCORES * DL, S // 2], BF16, tag=f"rsi{b}{h}",
                              name=f"rsi{b}{h}") for h in range(2)]
                   for b in range(B)]
            rso = [[dram.tile([DL, S // 2], BF16, tag=f"rso{b}{h}",
                              name=f"rso{b}{h}") for h in range(2)]
                   for b in range(B)]

            # --- small persistent tiles ---
            bqk_sb = sb.tile([128, 4], F32, tag="bias", bufs=1)
            nc.sync.dma_start(out=bqk_sb, in_=bqkd[:])
            bvf_sb = sb.tile([128, 2], F32, tag="bias2", bufs=1)
            nc.sync.dma_start(out=bvf_sb, in_=bvd[:])
            b1_sb = sb.tile([128, FCL], F32, tag="bias3", bufs=1)
            nc.sync.dma_start(out=b1_sb, in_=b1d[:])
            b2_sb = sb.tile([128, 1], F32, tag="bias4", bufs=1)
            nc.sync.dma_start(out=b2_sb, in_=b2d[:])

            # identity for PE transposes
            idb = sb.tile([128, 128], BF16, tag="idb", bufs=1)
            nc.vector.memset(idb, 1.0)
            nc.gpsimd.affine_select(out=idb, in_=idb,
                                    compare_op=ALU.is_equal, fill=0.0,
                                    base=0, channel_multiplier=-1,
                                    pattern=[[1, 128]])

            def load_w(dram_t, name, cols):
                w = sb.tile([128, KC, cols], BF16, tag=f"w_{name}", bufs=1,
                            name=name)
                nc.sync.dma_start(out=w, in_=dram_t[:])
                return w

            wq1 = load_w(wq1d, "wq1", DL)
            wk1 = load_w(wk1d, "wk1", DL)
            wv1 = load_w(wv1d, "wv1", DL)

            # persistent activation tiles
            kT1 = sb.tile([128, B, S], BF16, tag="kT1", bufs=1, name="kT1")
            vO1 = sb.tile([128, B, SC, HL, DK + 1], BF16, tag="vO1", bufs=1,
                          name="vO1")
            kT2 = sb.tile([128, B, S], BF16, tag="kT2", bufs=1, name="kT2")
            vO2 = sb.tile([128, B, SC, HL, DK + 1], BF16, tag="vO2", bufs=1,
                          name="vO2")
            qtsA = sb.tile([128, B, ST, 512], BF16, tag="qtsA", bufs=1,
                           name="qtsA")
            qtsC = sb.tile([128, B, ST, 512], BF16, tag="qtsC", bufs=1,
                           name="qtsC")
            # x ring: x1, x2, x3 (x3 reuses x1's buffer)
            x1 = sb.tile([128, B, S], F32, tag="xl", bufs=2, name="x1")
            x2 = sb.tile([128, B, S], F32, tag="xl", bufs=2, name="x2")

            def proj128(psrc, w, bias_col, out_ap):
                """One [128, 512] projection psum: out = W.T @ x (+bias)."""
                ps = pp.tile([128, 512], F32, tag="pp", name="ps")
                for k in range(KC):
                    nc.tensor.matmul(ps, w[:, k, :], psrc[:, k, :],
                                     start=(k == 0), stop=(k == KC - 1))
                nc.vector.tensor_scalar(out=out_ap, in0=ps,
                                        scalar1=bqk_sb[:, bias_col:bias_col + 1],
                                        scalar2=None, op0=ALU.add)

            def vproj(xs, wv, b, t, vO):
                """v^T projection + PE-transpose back to v-normal layout with
                an appended ones column."""
                vt = sb.tile([128, 512], BF16, tag="vt", bufs=2, name="vt")
                ps = pp.tile([128, 512], F32, tag="pp", name="ps")
                for k in range(KC):
                    nc.tensor.matmul(ps, wv[:, k, :], xs[:, k, :],
                                     start=(k == 0), stop=(k == KC - 1))
                nc.vector.tensor_copy(out=vt, in_=ps)
                for sc in range(4):
                    c = 4 * t + sc
                    tp = pp.tile([128, 1024], BF16, tag="pp", name="tp")
                    nc.tensor.transpose(tp[:, 0:128], vt[:, ts(sc, 128)], idb)
                    for h in range(HL):
                        nc.vector.tensor_copy(
                            out=vO[:, b, c, h, 0:DK],
                            in_=tp[:, ts(h, DK)])
                    nc.vector.tensor_scalar(
                        out=vO[:, b, c, :, DK:DK + 1],
                        in0=tp[:, 0:HL][:, :, None],
                        scalar1=0.0, scalar2=1.0,
                        op0=ALU.mult, op1=ALU.add)

            def qkv1_tile(b, t):
                xs = sb.tile([128, KC, 512], BF16, tag="xs", bufs=3,
                             name="xs")
                nc.sync.dma_start(out=xs, in_=xTd.ap()[:, b, t])
                proj128(xs, wq1, 0, qtsA[:, b, t, :])
                proj128(xs, wk1, 1, kT1[:, b, ts(t, 512)])
                vproj(xs, wv1, b, t, vO1)

            def kv2_tile(b, t):
                es = sb.tile([128, KC, 512], BF16, tag="xs", bufs=3,
                             name="es")
                nc.sync.dma_start(out=es, in_=encd.ap()[:, b, t])
                proj128(es, wk2, 3, kT2[:, b, ts(t, 512)])
                vproj(es, wv2, b, t, vO2)

            def attn_one(b, t, qt, kT, vO, xout, resid_sb, bv_col, causal,
                         stats=None):
                """One (b, sq-tile): E^T chunk tiles for both heads packed in
                one 2-bank psum (adjacent matmuls hit different PE row-groups
                and overlap), PV with ones-row, then a fused both-heads
                normalize + bias + residual into xout. The chunk loop is
                software-pipelined so E(c+1) sits AHEAD of PV(c) in the
                in-order PE queue and runs under exp(c). resid_sb is an SBUF
                [128, B, S] tile (or None to DMA the residual from resd)."""
                nchunks = (4 * t + 4) if causal else SC
                zps = [pz.tile([128, 512], F32, tag="pz", name="zps")
                       for _ in range(HL)]
                if resid_sb is None:
                    rs_ = sb.tile([128, 512], F32, tag="rs", bufs=2,
                                  name="rs_")
                    nc.sync.dma_start(out=rs_,
                                      in_=resd.ap()[:, b, ts(t, 512)])
                    resid_ap = rs_[:, :]
                else:
                    resid_ap = resid_sb[:, b, ts(t, 512)]

                def emit_E(c):
                    eps = pe.tile([128, 1024], F32, tag="pe", name="eps")
                    for h in range(HL):
                        hb = h * 64
                        nc.tensor.matmul(
                            eps[:, ts(h, 512)],
                            kT[hb:hb + 64, b, ts(c, 128)],
                            qt[hb:hb + 64, :],
                            start=True, stop=True)
                    return eps

                def emit_exp(c, eps):
                    et = sb.tile([128, 1024], BF16, tag="E", bufs=3,
                                 name="et")
                    nc.scalar.activation(out=et, in_=eps, func=AF.Exp,
                                         scale=float(SCALE))
                    if causal and c >= 4 * t:
                        j = c - 4 * t
                        for h in range(HL):
                            nc.gpsimd.affine_select(
                                out=et[:, ts(h, 512)],
                                in_=et[:, ts(h, 512)],
                                compare_op=ALU.is_ge,
                                fill=0.0, base=-(j * 128),
                                channel_multiplier=-1,
                                pattern=[[1, 512]])
                    return et

                def emit_PV(c, et):
                    for h in range(HL):
                        nc.tensor.matmul(
                            zps[h][0:DK + 1, :],
                            vO[:, b, c, h, :],
                            et[:, ts(h, 512)],
                            start=(c == 0),
                            stop=(c == nchunks - 1))

                et_prev = emit_exp(0, emit_E(0))
                for c in range(1, nchunks):
                    eps_c = emit_E(c)
                    emit_PV(c - 1, et_prev)
                    et_prev = emit_exp(c, eps_c)
                emit_PV(nchunks - 1, et_prev)
                # copy z (both heads) + denominators out of PSUM promptly so
                # the pz ring frees for the next row's PV accumulation
                zc = sb.tile([128, 512], F32, tag="zc", bufs=2, name="zc")
                dns = [sb.tile([1, 512], F32, tag="dn", bufs=4, name="dn")
                       for _ in range(HL)]
                for h in range(HL):
                    nc.vector.tensor_copy(out=zc[ts(h, 64), :],
                                          in_=zps[h][0:DK, :])
                    nc.vector.tensor_copy(out=dns[h],
                                          in_=zps[h][DK:DK + 1, :])
                # partition_broadcast mis-addresses outputs with a non-zero
                # base partition, so head 1 goes via a base-0 scratch tile
                rb = sb.tile([128, 512], F32, tag="rb", bufs=2, name="rb")
                nc.gpsimd.partition_broadcast(out_ap=rb[0:64, :],
                                              in_ap=dns[0])
                rb2 = sb.tile([64, 512], F32, tag="rb2", bufs=2, name="rb2")
                nc.gpsimd.partition_broadcast(out_ap=rb2, in_ap=dns[1])
                nc.vector.tensor_copy(out=rb[64:128, :], in_=rb2)
                nc.vector.reciprocal(rb, rb)
                nc.vector.tensor_mul(zc, zc, rb)
                nc.vector.scalar_tensor_tensor(
                    out=xout[:, b, ts(t, 512)], in0=zc,
                    scalar=bvf_sb[:, bv_col:bv_col + 1],
                    op0=ALU.add, in1=resid_ap, op1=ALU.add)
                if stats is not None:
                    nc.vector.bn_stats(out=stats[:, t, :],
                                       in_=xout[:, b, ts(t, 512)])

            def seqnorm_b(xt, b, stats=None):
                """In-place sequence-norm of one batch of a [128, B, S] f32
                tile. If per-tile bn_stats were precomputed, pass them in."""
                if stats is None:
                    stats = sb.tile([128, ST, 6], F32, tag="bnst", bufs=2,
                                    name="stats")
                    for g in range(ST):
                        nc.vector.bn_stats(out=stats[:, g, :],
                                           in_=xt[:, b, ts(g, 512)])
                mv = sb.tile([128, 2], F32, tag="bnmv", bufs=2, name="mv")
                nc.vector.bn_aggr(out=mv, in_=stats)
                r = sb.tile([128, 1], F32, tag="bnr", bufs=2, name="r")
                nc.vector.reciprocal(r, mv[:, 1:2])
                nc.vector.tensor_scalar(out=r, in0=r, scalar1=float(VARF),
                                        scalar2=None, op0=ALU.mult)
                mr = sb.tile([128, 1], F32, tag="bnmr", bufs=2, name="mr")
                nc.vector.scalar_tensor_tensor(
                    out=mr, in0=mv[:, 0:1], scalar=-1.0, op0=ALU.mult,
                    in1=r, op1=ALU.mult)
                nc.vector.scalar_tensor_tensor(
                    out=xt[:, b, :], in0=xt[:, b, :], scalar=r,
                    op0=ALU.mult, in1=mr.to_broadcast((128, S)),
                    op1=ALU.add)

            def bounce_ag(xt, b, bnc, full):
                """seqnorm'd batch b of xt -> casting DMA -> AllGather."""
                nc.gpsimd.dma_start(out=bnc[:], in_=xt[:, b, :])
                nc.gpsimd.collective_compute(
                    "AllGather", ALU.bypass, replica_groups=RG,
                    ins=[bnc[:]], outs=[full[:]])

            # ============ sublayer 1 + encoder K/V (interleaved) ===========
            wq2 = wk2 = wv2 = w1 = w2 = None
            for b in range(B):
                for t in range(ST):
                    qkv1_tile(b, t)
                if b == 0:
                    # big later-phase weight loads sit behind the first xs
                    # loads on the DMA queue so they don't delay the start
                    wk2 = load_w(wk2d, "wk2", DL)
                    wv2 = load_w(wv2d, "wv2", DL)
                    wq2 = load_w(wq2d, "wq2", DL)
                    w1 = sb.tile([128, KC, FFL], BF16, tag="w_w1", bufs=1,
                                 name="w1")
                    nc.sync.dma_start(out=w1, in_=w1d[:])
                    w2 = sb.tile([128, FCL, D], BF16, tag="w_w2", bufs=1,
                                 name="w2")
                    nc.sync.dma_start(out=w2, in_=w2d[:])
                stats1 = sb.tile([128, ST, 6], F32, tag="bnst", bufs=2,
                                 name="stats1")
                for t in range(ST):
                    attn_one(b, t, qtsA[:, b, t, :], kT1, vO1, x1,
                             None, bv_col=0, causal=True, stats=stats1)
                    kv2_tile(b, t)
                seqnorm_b(x1, b, stats1)
                bounce_ag(x1, b, x1b[b], x1f[b])

            if DBG:
                for b in range(B):
                    nc.sync.dma_start(out=dbg_x1[:, b, :], in_=x1[:, b, :])
                nc.sync.dma_start(out=dbg_kT1[:], in_=kT1)
                nc.gpsimd.dma_start(out=dbg_vO1[:], in_=vO1)
                nc.sync.dma_start(out=dbg_q1[:], in_=qtsA)

            # ================= sublayer 2: cross-attention =================
            for b in range(B):
                x1f_v = rview(x1f[b])
                for t in range(ST):
                    xs = sb.tile([128, KC, 512], BF16, tag="xsq", bufs=2,
                                 name="xs")
                    nc.sync.dma_start(out=xs, in_=x1f_v[:, :, ts(t, 512)])
                    proj128(xs, wq2, 2, qtsC[:, b, t, :])
                stats2 = sb.tile([128, ST, 6], F32, tag="bnst", bufs=2,
                                 name="stats2")
                for t in range(ST):
                    attn_one(b, t, qtsC[:, b, t, :], kT2, vO2, x2,
                             x1, bv_col=1, causal=False, stats=stats2)
                seqnorm_b(x2, b, stats2)
                bounce_ag(x2, b, x2b[b], x2f[b])

            if DBG:
                for b in range(B):
                    nc.sync.dma_start(out=dbg_x2[:, b, :], in_=x2[:, b, :])

            # ================= sublayer 3: FFN =============================
            for b in range(B):
                x2f_v = rview(x2f[b])
                for t in range(ST):
                    xs = sb.tile([128, KC, 512], BF16, tag="xsf", bufs=2,
                                 name="xs")
                    nc.sync.dma_start(out=xs, in_=x2f_v[:, :, ts(t, 512)])
                    hT = sb.tile([128, FCL, 512], BF16, tag="hT", bufs=2,
                                 name="hT")
                    for fc in range(FCL):
                        ps_h = pp.tile([128, 512], F32, tag="pp",
                                       name="ps_h")
                        for k in range(KC):
                            nc.tensor.matmul(ps_h,
                                             w1[:, k, ts(fc, 128)],
                                             xs[:, k, :],
                                             start=(k == 0),
                                             stop=(k == KC - 1))
                        nc.vector.tensor_scalar(
                            out=hT[:, fc, :], in0=ps_h,
                            scalar1=b1_sb[:, fc:fc + 1], scalar2=0.0,
                            op0=ALU.add, op1=ALU.max)
                    rsi_v = rview(rsi[b][t // 2])
                    for ec in range(KC):
                        ps_y = pp.tile([128, 512], F32, tag="pp",
                                       name="ps_y")
                        for fc in range(FCL):
                            nc.tensor.matmul(ps_y,
                                             w2[:, fc, ts(ec, 128)],
                                             hT[:, fc, :],
                                             start=(fc == 0),
                                             stop=(fc == FCL - 1))
                        ys = sb.tile([128, 512], BF16, tag="ys", bufs=2,
                                     name="ys")
                        nc.vector.tensor_copy(out=ys, in_=ps_y)
                        nc.sync.dma_start(out=rsi_v[:, ec, ts(t % 2, 512)],
                                          in_=ys)
                    if t % 2 == 1:
                        nc.gpsimd.collective_compute(
                            "ReduceScatter", ALU.add, replica_groups=RG,
                            ins=[rsi[b][t // 2][:]],
                            outs=[rso[b][t // 2][:]])

            # y + b2 + x2 residual (per quarter, overlapping the RS stream),
            # seqnorm, write out
            x3 = sb.tile([128, B, S], F32, tag="xl", bufs=2, name="x3")
            outv = outT[:].rearrange("(b p) s -> p b s", p=128)
            for b in range(B):
                stats3 = sb.tile([128, ST, 6], F32, tag="bnst", bufs=2,
                                 name="stats3")
                for half in range(2):
                    sl = ts(half, S // 2)
                    nc.gpsimd.dma_start(out=x3[:, b, sl],
                                        in_=rso[b][half][:])
                    nc.vector.scalar_tensor_tensor(
                        out=x3[:, b, sl], in0=x3[:, b, sl],
                        scalar=b2_sb[:, 0:1], op0=ALU.add,
                        in1=x2[:, b, sl], op1=ALU.add)
                    for g in range(2):
                        t = 2 * half + g
                        nc.vector.bn_stats(
                            out=stats3[:, t, :],
                            in_=x3[:, b, ts(t, 512)])
                seqnorm_b(x3, b, stats3)
                nc.sync.dma_start(out=outv[:, b, :], in_=x3[:, b, :])

    nc.compile()
    return nc


def _get_nc():
    global _CACHED_NC
    if _CACHED_NC is None:
        _CACHED_NC = _build()
    return _CACHED_NC


def _bf16(a):
    import ml_dtypes
    return np.asarray(a, np.float32).astype(ml_dtypes.bfloat16)


def _chunked(a):
    """[D, N] -> [128, D//128, N] with [p, c, n] = a[128c+p, n]."""
    d, n = a.shape
    return np.ascontiguousarray(
        a.reshape(d // 128, 128, n).transpose(1, 0, 2))


def _tiled_act(xT):
    """[B, D, S] -> [128, B, ST, KC, 512] bf16 with
    [p, b, t, c, j] = x[b, 128c+p, 512t+j]."""
    a = xT.reshape(B, KC, 128, ST, 512).transpose(2, 0, 3, 1, 4)
    return np.ascontiguousarray(_bf16(a))


def _make_in_maps(decoder_input, encode_input,
                  Wq1, Wk1, Wv1, bq1, bk1, bv1,
                  Wq2, Wk2, Wv2, bq2, bk2, bv2,
                  W1, b1, W2, b2):
    xT = np.ascontiguousarray(
        np.transpose(np.asarray(decoder_input, np.float32), (0, 2, 1)))
    eT = np.ascontiguousarray(
        np.transpose(np.asarray(encode_input, np.float32), (0, 2, 1)))
    xTd_all = _tiled_act(xT)
    encd_all = _tiled_act(eT)
    in_maps = []
    for r in range(NCORES):
        hs = slice(DL * r, DL * (r + 1))
        fs = slice(FFL * r, FFL * (r + 1))
        resd = np.ascontiguousarray(
            xT[:, hs, :].transpose(1, 0, 2))          # [128, B, S]
        bqk_arr = np.stack([bq1[hs], bk1[hs], bq2[hs], bk2[hs]],
                           axis=1).astype(np.float32)  # [128, 4]
        bv_arr = np.stack([bv1[hs], bv2[hs]],
                          axis=1).astype(np.float32)   # [128, 2]
        in_maps.append({
            "xTd": xTd_all,
            "encd": encd_all,
            "resd": resd,
            "wq1d": _bf16(_chunked(np.ascontiguousarray(Wq1[:, hs]))),
            "wk1d": _bf16(_chunked(np.ascontiguousarray(Wk1[:, hs]))),
            "wv1d": _bf16(_chunked(np.ascontiguousarray(Wv1[:, hs]))),
            "wq2d": _bf16(_chunked(np.ascontiguousarray(Wq2[:, hs]))),
            "wk2d": _bf16(_chunked(np.ascontiguousarray(Wk2[:, hs]))),
            "wv2d": _bf16(_chunked(np.ascontiguousarray(Wv2[:, hs]))),
            "w1d": _bf16(_chunked(np.ascontiguousarray(W1[:, fs]))),
            "w2d": _bf16(_chunked(np.ascontiguousarray(W2[fs, :]))),
            "bqkd": bqk_arr,
            "bvd": bv_arr,
            "b1d": np.ascontiguousarray(
                b1[fs].reshape(FCL, 128).T.astype(np.float32)),
            "b2d": np.ascontiguousarray(
                b2[hs].reshape(128, 1).astype(np.float32)),
        })
    return in_maps


def kernel(**inputs):
    nc = _get_nc()
    in_maps = _make_in_maps(**{k: np.asarray(v) for k, v in inputs.items()})
    res = run_bass_kernel_spmd(nc, in_maps, core_ids=list(range(NCORES)),
                               trace=False)
    out = np.empty((B, S, D), np.float32)
    for r in range(NCORES):
        hs = slice(DL * r, DL * (r + 1))
        o = res.results[r]["outT"]                     # [B*DL, S]
        for b in range(B):
            out[b, :, hs] = o[b * DL:(b + 1) * DL].T
    return out
